# revision 22
# baseline (speedup 1.0000x reference)
"""CfC cell (dense MLP) on 8 Trainium2 NeuronCores — data-parallel over batch.

Math (per sample):
    x  = concat(input, hx)                       # [1024]
    x1 = 1.7159*tanh(0.666*(Wb1 x + bb1))        # backbone 1, [1024]
    x2 = 1.7159*tanh(0.666*(Wb2 x1 + bb2))       # backbone 2, [1024]
    ff1 = tanh(W_ff1 x2 + b_ff1); ff2 = tanh(W_ff2 x2 + b_ff2)
    t   = sigmoid((W_ta x2 + b_ta)*ts + W_tb x2 + b_tb)
    out = ff1 + t*(ff2 - ff1)                    # [512]

Device layout: all activations live transposed as [feature_partition,
batch_free]; weights are host-pre-transposed to [K, N] so every GEMM is a
straight lhsT.T @ rhs chain with no on-device transposes. The 1.7159
LeCun gains are folded into the *next* layer's weights, the 0.666 input
scales into the ACT instruction's free affine + pre-scaled biases, so
each layer is exactly matmul-accumulate -> one ACT op. GEMM inputs are
fp16 (fp32 PSUM accumulation); everything elementwise is fp32.

Batch 8192 is split 1024/core across the 8 cores; weights are replicated.
"""
import os
from contextlib import ExitStack

import numpy as np

IN, HID, BB, B = 512, 512, 1024, 8192
N_CORES = 8
BL = B // N_CORES        # 1024 batch rows per core
K1 = IN + HID            # contraction dim of backbone layer 1 (== BB here)
KT = K1 // 128           # 8 k-tiles (also BB//128)
NB = BL // 512           # 2 batch chunks of 512 (PSUM bank = 512 fp32)
NT1 = BB // 128          # 8 output tiles for backbone layers
NTH = HID // 128         # 4 output tiles per head
LA, LBc = 1.7159, 0.666  # LeCun tanh gain / input scale

_F16 = np.float16

_cache: dict = {}

# Set by each kernel() call when tracing is enabled (BASS_KERNEL_TRACE=1).
LAST_EXEC_TIME_NS = None


def _install_ntff_shim():
    """Recreate the missing ``antenv.axon_hooks`` so trace=True works."""
    import sys, types, ctypes, contextlib

    if "antenv.axon_hooks" in sys.modules:
        return
    so_path = "/opt/axon/libaxon_pjrt.so"
    try:
        lib = ctypes.CDLL(so_path)
    except OSError:
        return
    if not hasattr(lib, "axon_start_nrt_profile"):
        return
    lib.axon_start_nrt_profile.argtypes = [ctypes.POINTER(ctypes.c_int64), ctypes.c_size_t]
    lib.axon_start_nrt_profile.restype = ctypes.c_int64
    lib.axon_stop_nrt_profile.argtypes = [ctypes.c_char_p]
    lib.axon_stop_nrt_profile.restype = ctypes.c_int64

    @contextlib.contextmanager
    def _hook(output_dir, device_ids):
        import jax

        jax.devices()
        if device_ids:
            ids = (ctypes.c_int64 * len(device_ids))(*device_ids)
            rc = lib.axon_start_nrt_profile(ids, len(device_ids))
        else:
            rc = lib.axon_start_nrt_profile(None, 0)
        if rc != 0:
            raise RuntimeError(f"axon_start_nrt_profile rc={rc}")
        try:
            yield
        finally:
            n = lib.axon_stop_nrt_profile(str(output_dir).encode())
            if n < 0:
                raise RuntimeError(f"axon_stop_nrt_profile rc={n}")

    mod = types.ModuleType("antenv.axon_hooks")
    mod.get_axon_ntff_profile_hook = lambda: _hook

    def set_axon_ntff_profile_hook(h):
        mod.get_axon_ntff_profile_hook = lambda: h

    mod.set_axon_ntff_profile_hook = set_axon_ntff_profile_hook
    sys.modules["antenv.axon_hooks"] = mod
    import antenv

    antenv.axon_hooks = mod


def _build():
    from concourse import bacc, tile, mybir

    f32 = mybir.dt.float32
    f16 = mybir.dt.float16
    Tanh = mybir.ActivationFunctionType.Tanh
    Sigm = mybir.ActivationFunctionType.Sigmoid

    nc = bacc.Bacc("TRN2", target_bir_lowering=False, debug=False, num_devices=N_CORES)

    # xt and w1t packed side by side so one DMA per k-tile delivers a
    # complete (xt_k, w1_k) operand pair; columns are [xt_b0 | w1 | xt_b1].
    l1p = nc.declare_dram_parameter("l1p", [K1, BL + BB], f16, isOutput=False)
    w2t = nc.declare_dram_parameter("w2t", [BB, BB], f16, isOutput=False)
    wht = nc.declare_dram_parameter("wht", [BB, 4 * HID], f16, isOutput=False)
    biases = nc.declare_dram_parameter("biases", [128, 36], f32, isOutput=False)
    tsb = nc.declare_dram_parameter("tsb", [128, BL], f32, isOutput=False)
    out = nc.declare_dram_parameter("out", [HID, BL], f32, isOutput=True)

    with tile.TileContext(nc) as tc, ExitStack() as ctx:
        sb = ctx.enter_context(tc.tile_pool(name="sb", bufs=1))
        tmp = ctx.enter_context(tc.tile_pool(name="tmp", bufs=2))
        ps = ctx.enter_context(tc.tile_pool(name="ps", bufs=8, space="PSUM"))

        # The sync HWDGE ring carries the big weight stream in consumption
        # order; the small biases/tsb transfers ride the otherwise-idle
        # scalar HWDGE ring so they arrive early without delaying l1p k0
        # (layer-1's PSUM evacuation blocks on the biases).
        # l1p columns are [xt_b0 | w1 | xt_b1]; the first DMA per k-tile is
        # just the 384 KB chunk-b0 matmuls need, so they start sooner and
        # stay PE-bound even when the DMA stream ramps slowly. Tile's
        # sub-tile dependency tracking keys the b0 matmuls on piece A only.
        l1_t = [sb.tile([128, BL + BB], f16, tag=f"l1{k}", name=f"l1{k}") for k in range(KT)]
        for k in range(KT):
            rows = slice(128 * k, 128 * (k + 1))
            if k < 3:
                # The first pairs pace the PE before its backlog builds up:
                # land [xt_b0 | w1 cols 0:512] (feeds matmuls n0-3) ahead of
                # [w1 cols 512:1024] (n4-7) to halve the arrival quantum.
                nc.sync.dma_start(l1_t[k][:, :1024], l1p[rows, :1024])
                nc.sync.dma_start(l1_t[k][:, 1024 : 512 + BB], l1p[rows, 1024 : 512 + BB])
            else:
                nc.sync.dma_start(l1_t[k][:, : 512 + BB], l1p[rows, : 512 + BB])
        for k in range(KT):
            rows = slice(128 * k, 128 * (k + 1))
            nc.sync.dma_start(l1_t[k][:, 512 + BB :], l1p[rows, 512 + BB :])
        bias_t = sb.tile([128, 36], f32, tag="bias")
        nc.scalar.dma_start(bias_t[:], biases[:])
        tsb_t = sb.tile([128, BL], f32, tag="tsb")
        nc.scalar.dma_start(tsb_t[:], tsb[:])
        w2_t = [sb.tile([128, BB], f16, tag=f"w2{k}", name=f"w2{k}") for k in range(KT)]
        wh_t = [sb.tile([128, 4 * HID], f16, tag=f"wh{k}", name=f"wh{k}") for k in range(KT)]
        for k in range(KT):
            nc.sync.dma_start(w2_t[k][:], w2t[128 * k : 128 * (k + 1), :])
        for k in range(KT):
            nc.sync.dma_start(wh_t[k][:], wht[128 * k : 128 * (k + 1), :])

        # HAM warm-up: dummy matmuls on a zeroed tile keep the PE busy while
        # the first real operands stream in, so real matmuls start at 2.4 GHz.
        warm = sb.tile([128, 512], f16, tag="warm")
        nc.vector.memset(warm[:], 0.0)
        wacc = ps.tile([128, 512], f32, tag="ps", name="warm_ps")
        N_WARM = 6
        for i in range(N_WARM):
            nc.tensor.matmul(
                wacc[:], warm[:, :128], warm[:], start=(i == 0), stop=(i == N_WARM - 1)
            )

        h1_t = [sb.tile([128, BL], f16, tag=f"h1{n}", name=f"h1{n}") for n in range(NT1)]
        h2_t = [sb.tile([128, BL], f16, tag=f"h2{n}", name=f"h2{n}") for n in range(NT1)]

        # backbone layer 1: h1 = tanh(0.666*(W1 x) + 0.666*bb1), fp16 out.
        # k-OUTER so each (xt_k, w1_k) pair is consumed for all 8 n-tiles the
        # moment its DMA lands — the PE streams during the input transfer
        # instead of stalling on the last k-tile. Needs 8 live PSUM banks.
        for b in range(NB):
            bsl = slice(512 * b, 512 * (b + 1))
            accs1 = [
                ps.tile([128, 512], f32, tag="ps", name=f"ps1_{b}_{n}")
                for n in range(NT1)
            ]
            # column layout of l1_t: [xt_b0 (512) | w1 (1024) | xt_b1 (512)]
            rhs_lo = 0 if b == 0 else 512 + BB
            for k in range(KT):
                for n in range(NT1):
                    nc.tensor.matmul(
                        accs1[n][:],
                        l1_t[k][:, 512 + 128 * n : 512 + 128 * (n + 1)],
                        l1_t[k][:, rhs_lo : rhs_lo + 512],
                        start=(k == 0),
                        stop=(k == KT - 1),
                    )
            for n in range(NT1):
                nc.scalar.activation(
                    h1_t[n][:, bsl], accs1[n][:], Tanh, bias=bias_t[:, n : n + 1], scale=LBc
                )

        # backbone layer 2 (1.7159 folded into w2t on host)
        for b in range(NB):
            bsl = slice(512 * b, 512 * (b + 1))
            for n in range(NT1):
                acc = ps.tile([128, 512], f32, tag="ps", name=f"ps2_{b}_{n}")
                for k in range(KT):
                    nc.tensor.matmul(
                        acc[:],
                        w2_t[k][:, 128 * n : 128 * (n + 1)],
                        h1_t[k][:, bsl],
                        start=(k == 0),
                        stop=(k == KT - 1),
                    )
                nc.scalar.activation(
                    h2_t[n][:, bsl], acc[:], Tanh, bias=bias_t[:, 8 + n : 9 + n], scale=LBc
                )

        # heads: order ta, tb, ff1, ff2 so the sigmoid chain overlaps the
        # ff GEMMs and only ACT(ff2) -> mul -> add trail the last matmul.
        # Uses 1-t = sigmoid(-x) to form w = (1-t) early.
        def head_group(b, n, lo, w, sub):
            """One head pipeline over batch cols [512*b+lo, 512*b+lo+w)."""
            bsl = slice(512 * b + lo, 512 * b + lo + w)

            def bc(h):
                c = 16 + 4 * h + n
                return bias_t[:, c : c + 1]

            def head_mms(h, acc):
                col = 512 * h + 128 * n
                for k in range(KT):
                    nc.tensor.matmul(
                        acc[:],
                        wh_t[k][:, col : col + 128],
                        h2_t[k][:, bsl],
                        start=(k == 0),
                        stop=(k == KT - 1),
                    )

            sfx = f"_{b}_{n}_{sub}"
            ta_ps = ps.tile([128, w], f32, tag="ps", name="ps_ta" + sfx)
            head_mms(2, ta_ps)
            tb_ps = ps.tile([128, w], f32, tag="ps", name="ps_tb" + sfx)
            head_mms(3, tb_ps)

            v1 = tmp.tile([128, w], f32, tag="v1", name="v1" + sfx)
            nc.vector.tensor_scalar_add(v1[:], ta_ps[:], bc(2))  # t_a
            v2 = tmp.tile([128, w], f32, tag="v2", name="v2" + sfx)
            nc.vector.tensor_mul(v2[:], v1[:], tsb_t[:, bsl])  # t_a*ts
            v3 = tmp.tile([128, w], f32, tag="v3", name="v3" + sfx)
            nc.vector.tensor_add(v3[:], v2[:], tb_ps[:])  # + (W_tb x2)
            tpos = tmp.tile([128, w], f32, tag="tpos", name="tpos" + sfx)
            nc.scalar.activation(tpos[:], v3[:], Sigm, bias=bc(3))  # t
            tneg = tmp.tile([128, w], f32, tag="tneg", name="tneg" + sfx)
            wcol = 32 + n
            nc.scalar.activation(
                tneg[:], v3[:], Sigm, bias=bias_t[:, wcol : wcol + 1], scale=-1.0
            )  # 1-t = sigmoid(-(v3 + b_tb))

            ff1_ps = ps.tile([128, w], f32, tag="ps", name="ps_ff1" + sfx)
            head_mms(0, ff1_ps)
            ff1 = tmp.tile([128, w], f32, tag="ff1", name="ff1" + sfx)
            nc.scalar.activation(ff1[:], ff1_ps[:], Tanh, bias=bc(0))
            u = tmp.tile([128, w], f32, tag="u", name="u" + sfx)
            nc.vector.tensor_mul(u[:], tneg[:], ff1[:])  # (1-t)*ff1

            ff2_ps = ps.tile([128, w], f32, tag="ps", name="ps_ff2" + sfx)
            head_mms(1, ff2_ps)
            ff2 = tmp.tile([128, w], f32, tag="ff2", name="ff2" + sfx)
            nc.scalar.activation(ff2[:], ff2_ps[:], Tanh, bias=bc(1))
            m2 = tmp.tile([128, w], f32, tag="m2", name="m2" + sfx)
            nc.vector.tensor_mul(m2[:], tpos[:], ff2[:])  # t*ff2
            o = tmp.tile([128, w], f32, tag="o", name="o" + sfx)
            nc.vector.tensor_add(o[:], u[:], m2[:])
            nc.sync.dma_start(out[128 * n : 128 * (n + 1), bsl], o[:])

        for b in range(NB):
            for n in range(NTH):
                if b == NB - 1 and n == NTH - 1:
                    # Final group: two half-batch pipelines, so the first
                    # half's epilogue overlaps the second half's GEMMs and
                    # only a 256-wide ACT->mul->add->DMA trails the last MM.
                    head_group(b, n, 0, 256, 0)
                    head_group(b, n, 256, 256, 1)
                else:
                    head_group(b, n, 0, 512, 0)

    nc.finalize()
    return nc


def _prep_shared(Wb1, bb1, Wb2, bb2, W_ff1, b_ff1, W_ff2, b_ff2, W_ta, b_ta, W_tb, b_tb):
    """Host-side weight layout: transpose to [K, N], fold LeCun gains."""
    w1t = Wb1.T.astype(_F16)
    w2t = np.ascontiguousarray((LA * Wb2).T).astype(_F16)
    wht = np.ascontiguousarray(
        np.concatenate([(LA * W).T for W in (W_ff1, W_ff2, W_ta, W_tb)], axis=1)
    ).astype(_F16)

    biases = np.zeros((128, 36), np.float32)
    biases[:, 0:8] = (LBc * bb1).reshape(8, 128).T
    biases[:, 8:16] = (LBc * bb2).reshape(8, 128).T
    for h, bh in enumerate((b_ff1, b_ff2, b_ta, b_tb)):
        biases[:, 16 + 4 * h : 20 + 4 * h] = bh.reshape(4, 128).T
    biases[:, 32:36] = (-b_tb).reshape(4, 128).T
    return w1t, w2t, wht, biases


def kernel(input, hx, ts, Wb1, bb1, Wb2, bb2, W_ff1, b_ff1, W_ff2, b_ff2, W_ta, b_ta, W_tb, b_tb):
    global LAST_EXEC_TIME_NS
    from concourse.bass_utils import run_bass_kernel_spmd

    trace = os.environ.get("BASS_KERNEL_TRACE", "0") == "1"
    if trace:
        _install_ntff_shim()

    if "nc" not in _cache:
        _cache["nc"] = _build()
    nc = _cache["nc"]

    input = np.asarray(input, np.float32)
    hx = np.asarray(hx, np.float32)
    ts = np.asarray(ts, np.float32)
    w1t, w2t, wht, biases = _prep_shared(
        np.asarray(Wb1, np.float32), np.asarray(bb1, np.float32),
        np.asarray(Wb2, np.float32), np.asarray(bb2, np.float32),
        np.asarray(W_ff1, np.float32), np.asarray(b_ff1, np.float32),
        np.asarray(W_ff2, np.float32), np.asarray(b_ff2, np.float32),
        np.asarray(W_ta, np.float32), np.asarray(b_ta, np.float32),
        np.asarray(W_tb, np.float32), np.asarray(b_tb, np.float32),
    )

    in_maps = []
    for c in range(N_CORES):
        sl = slice(c * BL, (c + 1) * BL)
        x_c = np.concatenate([input[sl], hx[sl]], axis=1)  # [BL, K1]
        xt_c = x_c.T.astype(_F16)  # [K1, BL]
        # pack [x^T(b0) | Wb1^T | x^T(b1)] so the first DMA per k-tile
        # carries exactly what chunk-b0's matmuls need
        l1p_c = np.concatenate([xt_c[:, :512], w1t, xt_c[:, 512:]], axis=1)
        tsb_c = np.ascontiguousarray(
            np.broadcast_to(ts[sl].reshape(1, BL), (128, BL))
        ).astype(np.float32)
        in_maps.append(
            {
                "l1p": l1p_c,
                "w2t": w2t,
                "wht": wht,
                "biases": biases,
                "tsb": tsb_c,
            }
        )

    res = run_bass_kernel_spmd(nc, in_maps, list(range(N_CORES)), trace=trace)
    LAST_EXEC_TIME_NS = res.exec_time_ns

    full = np.empty((B, HID), np.float32)
    for c in range(N_CORES):
        full[c * BL : (c + 1) * BL] = res.results[c]["out"].T
    return full


# revision 23
# speedup vs baseline: 1.0184x; 1.0184x over previous
"""CfC cell (dense MLP) on 8 Trainium2 NeuronCores — data-parallel over batch.

Math (per sample):
    x  = concat(input, hx)                       # [1024]
    x1 = 1.7159*tanh(0.666*(Wb1 x + bb1))        # backbone 1, [1024]
    x2 = 1.7159*tanh(0.666*(Wb2 x1 + bb2))       # backbone 2, [1024]
    ff1 = tanh(W_ff1 x2 + b_ff1); ff2 = tanh(W_ff2 x2 + b_ff2)
    t   = sigmoid((W_ta x2 + b_ta)*ts + W_tb x2 + b_tb)
    out = ff1 + t*(ff2 - ff1)                    # [512]

Device layout: all activations live transposed as [feature_partition,
batch_free]; weights are host-pre-transposed to [K, N] so every GEMM is a
straight lhsT.T @ rhs chain with no on-device transposes. The 1.7159
LeCun gains are folded into the *next* layer's weights, the 0.666 input
scales into the ACT instruction's free affine + pre-scaled biases, so
each layer is exactly matmul-accumulate -> one ACT op. GEMM inputs are
fp16 (fp32 PSUM accumulation); everything elementwise is fp32.

Batch 8192 is split 1024/core across the 8 cores; weights are replicated.
"""
import os
from contextlib import ExitStack

import numpy as np

IN, HID, BB, B = 512, 512, 1024, 8192
N_CORES = 8
BL = B // N_CORES        # 1024 batch rows per core
K1 = IN + HID            # contraction dim of backbone layer 1 (== BB here)
KT = K1 // 128           # 8 k-tiles (also BB//128)
NB = BL // 512           # 2 batch chunks of 512 (PSUM bank = 512 fp32)
NT1 = BB // 128          # 8 output tiles for backbone layers
NTH = HID // 128         # 4 output tiles per head
LA, LBc = 1.7159, 0.666  # LeCun tanh gain / input scale

_F16 = np.float16

_cache: dict = {}

# Set by each kernel() call when tracing is enabled (BASS_KERNEL_TRACE=1).
LAST_EXEC_TIME_NS = None


def _install_ntff_shim():
    """Recreate the missing ``antenv.axon_hooks`` so trace=True works."""
    import sys, types, ctypes, contextlib

    if "antenv.axon_hooks" in sys.modules:
        return
    so_path = "/opt/axon/libaxon_pjrt.so"
    try:
        lib = ctypes.CDLL(so_path)
    except OSError:
        return
    if not hasattr(lib, "axon_start_nrt_profile"):
        return
    lib.axon_start_nrt_profile.argtypes = [ctypes.POINTER(ctypes.c_int64), ctypes.c_size_t]
    lib.axon_start_nrt_profile.restype = ctypes.c_int64
    lib.axon_stop_nrt_profile.argtypes = [ctypes.c_char_p]
    lib.axon_stop_nrt_profile.restype = ctypes.c_int64

    @contextlib.contextmanager
    def _hook(output_dir, device_ids):
        import jax

        jax.devices()
        if device_ids:
            ids = (ctypes.c_int64 * len(device_ids))(*device_ids)
            rc = lib.axon_start_nrt_profile(ids, len(device_ids))
        else:
            rc = lib.axon_start_nrt_profile(None, 0)
        if rc != 0:
            raise RuntimeError(f"axon_start_nrt_profile rc={rc}")
        try:
            yield
        finally:
            n = lib.axon_stop_nrt_profile(str(output_dir).encode())
            if n < 0:
                raise RuntimeError(f"axon_stop_nrt_profile rc={n}")

    mod = types.ModuleType("antenv.axon_hooks")
    mod.get_axon_ntff_profile_hook = lambda: _hook

    def set_axon_ntff_profile_hook(h):
        mod.get_axon_ntff_profile_hook = lambda: h

    mod.set_axon_ntff_profile_hook = set_axon_ntff_profile_hook
    sys.modules["antenv.axon_hooks"] = mod
    import antenv

    antenv.axon_hooks = mod


def _build():
    from concourse import bacc, tile, mybir

    f32 = mybir.dt.float32
    f16 = mybir.dt.float16
    Tanh = mybir.ActivationFunctionType.Tanh
    Sigm = mybir.ActivationFunctionType.Sigmoid

    nc = bacc.Bacc("TRN2", target_bir_lowering=False, debug=False, num_devices=N_CORES)

    # xt and w1t packed side by side so one DMA per k-tile delivers a
    # complete (xt_k, w1_k) operand pair; columns are [xt_b0 | w1 | xt_b1].
    l1p = nc.declare_dram_parameter("l1p", [K1, BL + BB], f16, isOutput=False)
    w2t = nc.declare_dram_parameter("w2t", [BB, BB], f16, isOutput=False)
    wht = nc.declare_dram_parameter("wht", [BB, 4 * HID], f16, isOutput=False)
    biases = nc.declare_dram_parameter("biases", [128, 36], f32, isOutput=False)
    tsb = nc.declare_dram_parameter("tsb", [128, BL], f32, isOutput=False)
    out = nc.declare_dram_parameter("out", [HID, BL], f32, isOutput=True)

    with tile.TileContext(nc) as tc, ExitStack() as ctx:
        sb = ctx.enter_context(tc.tile_pool(name="sb", bufs=1))
        tmp = ctx.enter_context(tc.tile_pool(name="tmp", bufs=2))
        ps = ctx.enter_context(tc.tile_pool(name="ps", bufs=8, space="PSUM"))

        # The sync HWDGE ring carries the big weight stream in consumption
        # order; the small biases/tsb transfers ride the otherwise-idle
        # scalar HWDGE ring so they arrive early without delaying l1p k0
        # (layer-1's PSUM evacuation blocks on the biases).
        # l1p columns are [xt_b0 | w1 | xt_b1]; the first DMA per k-tile is
        # just the 384 KB chunk-b0 matmuls need, so they start sooner and
        # stay PE-bound even when the DMA stream ramps slowly. Tile's
        # sub-tile dependency tracking keys the b0 matmuls on piece A only.
        l1_t = [sb.tile([128, BL + BB], f16, tag=f"l1{k}", name=f"l1{k}") for k in range(KT)]
        for k in range(KT):
            rows = slice(128 * k, 128 * (k + 1))
            nc.sync.dma_start(l1_t[k][:, : 512 + BB], l1p[rows, : 512 + BB])
        for k in range(KT):
            rows = slice(128 * k, 128 * (k + 1))
            nc.sync.dma_start(l1_t[k][:, 512 + BB :], l1p[rows, 512 + BB :])
        bias_t = sb.tile([128, 36], f32, tag="bias")
        nc.scalar.dma_start(bias_t[:], biases[:])
        tsb_t = sb.tile([128, BL], f32, tag="tsb")
        nc.scalar.dma_start(tsb_t[:], tsb[:])
        w2_t = [sb.tile([128, BB], f16, tag=f"w2{k}", name=f"w2{k}") for k in range(KT)]
        wh_t = [sb.tile([128, 4 * HID], f16, tag=f"wh{k}", name=f"wh{k}") for k in range(KT)]
        for k in range(KT):
            nc.sync.dma_start(w2_t[k][:], w2t[128 * k : 128 * (k + 1), :])
        for k in range(KT):
            nc.sync.dma_start(wh_t[k][:], wht[128 * k : 128 * (k + 1), :])

        # HAM warm-up: dummy matmuls on a zeroed tile keep the PE busy while
        # the first real operands stream in, so real matmuls start at 2.4 GHz.
        warm = sb.tile([128, 512], f16, tag="warm")
        nc.vector.memset(warm[:], 0.0)
        wacc = ps.tile([128, 512], f32, tag="ps", name="warm_ps")
        N_WARM = 6
        for i in range(N_WARM):
            nc.tensor.matmul(
                wacc[:], warm[:, :128], warm[:], start=(i == 0), stop=(i == N_WARM - 1)
            )

        h1_t = [sb.tile([128, BL], f16, tag=f"h1{n}", name=f"h1{n}") for n in range(NT1)]
        h2_t = [sb.tile([128, BL], f16, tag=f"h2{n}", name=f"h2{n}") for n in range(NT1)]

        # backbone layer 1: h1 = tanh(0.666*(W1 x) + 0.666*bb1), fp16 out.
        # k-OUTER so each (xt_k, w1_k) pair is consumed for all 8 n-tiles the
        # moment its DMA lands — the PE streams during the input transfer
        # instead of stalling on the last k-tile. Needs 8 live PSUM banks.
        for b in range(NB):
            bsl = slice(512 * b, 512 * (b + 1))
            accs1 = [
                ps.tile([128, 512], f32, tag="ps", name=f"ps1_{b}_{n}")
                for n in range(NT1)
            ]
            # column layout of l1_t: [xt_b0 (512) | w1 (1024) | xt_b1 (512)]
            rhs_lo = 0 if b == 0 else 512 + BB
            for k in range(KT):
                for n in range(NT1):
                    nc.tensor.matmul(
                        accs1[n][:],
                        l1_t[k][:, 512 + 128 * n : 512 + 128 * (n + 1)],
                        l1_t[k][:, rhs_lo : rhs_lo + 512],
                        start=(k == 0),
                        stop=(k == KT - 1),
                    )
            for n in range(NT1):
                nc.scalar.activation(
                    h1_t[n][:, bsl], accs1[n][:], Tanh, bias=bias_t[:, n : n + 1], scale=LBc
                )

        # backbone layer 2 (1.7159 folded into w2t on host)
        for b in range(NB):
            bsl = slice(512 * b, 512 * (b + 1))
            for n in range(NT1):
                acc = ps.tile([128, 512], f32, tag="ps", name=f"ps2_{b}_{n}")
                for k in range(KT):
                    nc.tensor.matmul(
                        acc[:],
                        w2_t[k][:, 128 * n : 128 * (n + 1)],
                        h1_t[k][:, bsl],
                        start=(k == 0),
                        stop=(k == KT - 1),
                    )
                nc.scalar.activation(
                    h2_t[n][:, bsl], acc[:], Tanh, bias=bias_t[:, 8 + n : 9 + n], scale=LBc
                )

        # heads: order ta, tb, ff1, ff2 so the sigmoid chain overlaps the
        # ff GEMMs and only ACT(ff2) -> mul -> add trail the last matmul.
        # Uses 1-t = sigmoid(-x) to form w = (1-t) early.
        def head_group(b, n, lo, w, sub):
            """One head pipeline over batch cols [512*b+lo, 512*b+lo+w)."""
            bsl = slice(512 * b + lo, 512 * b + lo + w)

            def bc(h):
                c = 16 + 4 * h + n
                return bias_t[:, c : c + 1]

            def head_mms(h, acc):
                col = 512 * h + 128 * n
                for k in range(KT):
                    nc.tensor.matmul(
                        acc[:],
                        wh_t[k][:, col : col + 128],
                        h2_t[k][:, bsl],
                        start=(k == 0),
                        stop=(k == KT - 1),
                    )

            sfx = f"_{b}_{n}_{sub}"
            ta_ps = ps.tile([128, w], f32, tag="ps", name="ps_ta" + sfx)
            head_mms(2, ta_ps)
            tb_ps = ps.tile([128, w], f32, tag="ps", name="ps_tb" + sfx)
            head_mms(3, tb_ps)

            v1 = tmp.tile([128, w], f32, tag="v1", name="v1" + sfx)
            nc.vector.tensor_scalar_add(v1[:], ta_ps[:], bc(2))  # t_a
            v2 = tmp.tile([128, w], f32, tag="v2", name="v2" + sfx)
            nc.vector.tensor_mul(v2[:], v1[:], tsb_t[:, bsl])  # t_a*ts
            v3 = tmp.tile([128, w], f32, tag="v3", name="v3" + sfx)
            nc.vector.tensor_add(v3[:], v2[:], tb_ps[:])  # + (W_tb x2)
            tpos = tmp.tile([128, w], f32, tag="tpos", name="tpos" + sfx)
            nc.scalar.activation(tpos[:], v3[:], Sigm, bias=bc(3))  # t
            tneg = tmp.tile([128, w], f32, tag="tneg", name="tneg" + sfx)
            wcol = 32 + n
            nc.scalar.activation(
                tneg[:], v3[:], Sigm, bias=bias_t[:, wcol : wcol + 1], scale=-1.0
            )  # 1-t = sigmoid(-(v3 + b_tb))

            ff1_ps = ps.tile([128, w], f32, tag="ps", name="ps_ff1" + sfx)
            head_mms(0, ff1_ps)
            ff1 = tmp.tile([128, w], f32, tag="ff1", name="ff1" + sfx)
            nc.scalar.activation(ff1[:], ff1_ps[:], Tanh, bias=bc(0))
            u = tmp.tile([128, w], f32, tag="u", name="u" + sfx)
            nc.vector.tensor_mul(u[:], tneg[:], ff1[:])  # (1-t)*ff1

            ff2_ps = ps.tile([128, w], f32, tag="ps", name="ps_ff2" + sfx)
            head_mms(1, ff2_ps)
            ff2 = tmp.tile([128, w], f32, tag="ff2", name="ff2" + sfx)
            nc.scalar.activation(ff2[:], ff2_ps[:], Tanh, bias=bc(1))
            m2 = tmp.tile([128, w], f32, tag="m2", name="m2" + sfx)
            nc.vector.tensor_mul(m2[:], tpos[:], ff2[:])  # t*ff2
            o = tmp.tile([128, w], f32, tag="o", name="o" + sfx)
            nc.vector.tensor_add(o[:], u[:], m2[:])
            nc.sync.dma_start(out[128 * n : 128 * (n + 1), bsl], o[:])

        for b in range(NB):
            for n in range(NTH):
                if b == NB - 1 and n == NTH - 1:
                    # Final group: two half-batch pipelines, so the first
                    # half's epilogue overlaps the second half's GEMMs and
                    # only a 256-wide ACT->mul->add->DMA trails the last MM.
                    head_group(b, n, 0, 256, 0)
                    head_group(b, n, 256, 256, 1)
                else:
                    head_group(b, n, 0, 512, 0)

    nc.finalize()
    return nc


def _prep_shared(Wb1, bb1, Wb2, bb2, W_ff1, b_ff1, W_ff2, b_ff2, W_ta, b_ta, W_tb, b_tb):
    """Host-side weight layout: transpose to [K, N], fold LeCun gains."""
    w1t = Wb1.T.astype(_F16)
    w2t = np.ascontiguousarray((LA * Wb2).T).astype(_F16)
    wht = np.ascontiguousarray(
        np.concatenate([(LA * W).T for W in (W_ff1, W_ff2, W_ta, W_tb)], axis=1)
    ).astype(_F16)

    biases = np.zeros((128, 36), np.float32)
    biases[:, 0:8] = (LBc * bb1).reshape(8, 128).T
    biases[:, 8:16] = (LBc * bb2).reshape(8, 128).T
    for h, bh in enumerate((b_ff1, b_ff2, b_ta, b_tb)):
        biases[:, 16 + 4 * h : 20 + 4 * h] = bh.reshape(4, 128).T
    biases[:, 32:36] = (-b_tb).reshape(4, 128).T
    return w1t, w2t, wht, biases


def kernel(input, hx, ts, Wb1, bb1, Wb2, bb2, W_ff1, b_ff1, W_ff2, b_ff2, W_ta, b_ta, W_tb, b_tb):
    global LAST_EXEC_TIME_NS
    from concourse.bass_utils import run_bass_kernel_spmd

    trace = os.environ.get("BASS_KERNEL_TRACE", "0") == "1"
    if trace:
        _install_ntff_shim()

    if "nc" not in _cache:
        _cache["nc"] = _build()
    nc = _cache["nc"]

    input = np.asarray(input, np.float32)
    hx = np.asarray(hx, np.float32)
    ts = np.asarray(ts, np.float32)
    w1t, w2t, wht, biases = _prep_shared(
        np.asarray(Wb1, np.float32), np.asarray(bb1, np.float32),
        np.asarray(Wb2, np.float32), np.asarray(bb2, np.float32),
        np.asarray(W_ff1, np.float32), np.asarray(b_ff1, np.float32),
        np.asarray(W_ff2, np.float32), np.asarray(b_ff2, np.float32),
        np.asarray(W_ta, np.float32), np.asarray(b_ta, np.float32),
        np.asarray(W_tb, np.float32), np.asarray(b_tb, np.float32),
    )

    in_maps = []
    for c in range(N_CORES):
        sl = slice(c * BL, (c + 1) * BL)
        x_c = np.concatenate([input[sl], hx[sl]], axis=1)  # [BL, K1]
        xt_c = x_c.T.astype(_F16)  # [K1, BL]
        # pack [x^T(b0) | Wb1^T | x^T(b1)] so the first DMA per k-tile
        # carries exactly what chunk-b0's matmuls need
        l1p_c = np.concatenate([xt_c[:, :512], w1t, xt_c[:, 512:]], axis=1)
        tsb_c = np.ascontiguousarray(
            np.broadcast_to(ts[sl].reshape(1, BL), (128, BL))
        ).astype(np.float32)
        in_maps.append(
            {
                "l1p": l1p_c,
                "w2t": w2t,
                "wht": wht,
                "biases": biases,
                "tsb": tsb_c,
            }
        )

    res = run_bass_kernel_spmd(nc, in_maps, list(range(N_CORES)), trace=trace)
    LAST_EXEC_TIME_NS = res.exec_time_ns

    full = np.empty((B, HID), np.float32)
    for c in range(N_CORES):
        full[c * BL : (c + 1) * BL] = res.results[c]["out"].T
    return full
